# revision 1
# baseline (speedup 1.0000x reference)
"""Trainium2 Bass kernel for InteractorwoLSTM additive attention.

out[b,t,:] = alpha[b,t,:] @ h_s[b]  with
  beta[b,t,n] = W_w . tanh(h_s[b,n]@W_S + b_S + h_v[b,t]@W_V + b_V) + b_w
  alpha = masked-softmax(beta) per reference semantics.

Sharding: data-parallel over batch B=32 across 8 cores (4 batches/core);
all weights replicated.

v2 design (per core):
  - All input transposes done HOST-side (hvT/hsT/weights pre-chunked,
    bf16) -> no PE transposes, no psum copies for inputs.
  - Projections weight-stationary in bf16 (FWL): VT[d,t] and
    ST'[d,(b,n)] with (b_S+b_V) bias folded in via a K=1 rank-1 matmul.
  - e_pre[d,n,t] = VT (+bcast over n) + ST' (+bcast over t) on DVE at
    2x_1P: bf16 operands, ST duplicated over t-PAIRS so every operand's
    innermost AP dim is step-1 x>=2.
  - e = tanh(e_pre) on ACT (bf16, table shared with exp).
  - beta[t,n]: per (n,c) matmul with lhsT = e[:,c,n,:] (bf16 -> fast
    weight load), rhs = Ww chunk [128,1] -> psum [128t, 30n] directly.
  - masked softmax reads beta straight from PSUM; denominator folded
    into the output copy (tensor_scalar mult by reciprocal).
  - out[b] = qT.T @ h_s[b] (fp32), scaled by recip on the psum->sbuf
    copy, then DMA to DRAM.
"""

import os
import numpy as np

B, T, N = 32, 128, 30
D = 512
NCORES = 8
BPC = B // NCORES  # batches per core
NC = D // 128  # 4 chunks of 128 along D

_CACHE = {}


def _build(variant: str = "bf16pair"):
    import concourse.bacc as bacc
    import concourse.tile as tile
    from concourse import mybir
    from concourse.masks import make_identity

    f32 = mybir.dt.float32
    bf16 = mybir.dt.bfloat16
    debug = variant == "dbgf32"
    if variant in ("f32plain", "dbgf32"):
        bf16 = f32  # everything fp32; broadcast add without pair trick
    use_pair = variant == "bf16pair"

    nc = bacc.Bacc(
        "TRN2",
        target_bir_lowering=False,
        debug=False,
        enable_asserts=True,
        num_devices=NCORES,
    )

    # ---- DRAM I/O (all host-prepped layouts) ----
    hvT_d = nc.dram_tensor("hvT", [128, BPC, NC, 128], bf16, kind="ExternalInput").ap()
    hsT_d = nc.dram_tensor("hsT", [128, NC, BPC * N], bf16, kind="ExternalInput").ap()
    hs_d = nc.dram_tensor("hs", [N, BPC, D], bf16, kind="ExternalInput").ap()
    WS_d = nc.dram_tensor("WS", [128, NC, NC, 128], bf16, kind="ExternalInput").ap()
    WV_d = nc.dram_tensor("WV", [128, NC, NC, 128], bf16, kind="ExternalInput").ap()
    Ww_d = nc.dram_tensor("Ww", [128, NC], bf16, kind="ExternalInput").ap()
    bSV_d = nc.dram_tensor("bSV", [1, D], bf16, kind="ExternalInput").ap()
    bw_d = nc.dram_tensor("bw", [128, 1], f32, kind="ExternalInput").ap()
    mask_d = nc.dram_tensor("mask", [128, BPC, N], f32, kind="ExternalInput").ap()
    # nmask[b] = N - lengths[b]: masked entries contribute exactly exp(0)=1
    # to Z1, so Qs = Z1 - nmask (h_s arrives with masked rows zeroed)
    nmask_d = nc.dram_tensor("nmask", [128, BPC], f32, kind="ExternalInput").ap()
    # output in bf16 (host casts back to f32): halves the tail DMA; adds
    # ~0.2% norm error on top of 0.18%, still far under the 2e-2 gate
    out_dt = f32 if debug else bf16
    out_d = nc.dram_tensor("out", [BPC, T, D], out_dt, kind="ExternalOutput").ap()

    with tile.TileContext(nc) as tc:
        with (
            tc.tile_pool(name="const", bufs=1) as const,
            tc.tile_pool(name="epre", bufs=4) as eprep,
            tc.tile_pool(name="epre2", bufs=3) as eprep2,
            tc.tile_pool(name="ebig", bufs=2 if use_pair else 1) as ebigp,
            tc.tile_pool(name="soft", bufs=3) as softp,
            tc.tile_pool(name="outp", bufs=3) as outp,
            tc.tile_pool(name="pv", bufs=2, space="PSUM") as pvp,
            tc.tile_pool(name="ps", bufs=2, space="PSUM") as psp,
            tc.tile_pool(name="pbeta", bufs=2, space="PSUM") as pbetap,
            tc.tile_pool(name="pqt", bufs=1, space="PSUM") as pqtp,
            tc.tile_pool(name="pfin", bufs=1, space="PSUM") as pfinp,
        ):
            # ---- constants / weights (spread DMAs across engine queues so
            # dispatch doesn't serialize; proj-critical tensors first) ----
            WS_sb = const.tile([128, NC, NC, 128], bf16)
            nc.sync.dma_start(out=WS_sb[:], in_=WS_d)
            hsT_sb = const.tile([128, NC, BPC * N], bf16)
            nc.scalar.dma_start(out=hsT_sb[:], in_=hsT_d)
            bSV_sb = const.tile([1, D], bf16)
            nc.scalar.dma_start(out=bSV_sb[:], in_=bSV_d)
            WV_sb = const.tile([128, NC, NC, 128], bf16)
            nc.gpsimd.dma_start(out=WV_sb[:], in_=WV_d)
            hvT_sb = const.tile([128, BPC, NC, 128], bf16)
            nc.scalar.dma_start(out=hvT_sb[:], in_=hvT_d)
            Ww_sb = const.tile([128, NC], bf16)
            nc.sync.dma_start(out=Ww_sb[:], in_=Ww_d)
            bw_sb = const.tile([128, 1], f32)
            nc.sync.dma_start(out=bw_sb[:], in_=bw_d)
            mask_sb = const.tile([128, BPC, N], f32)
            nc.gpsimd.dma_start(out=mask_sb[:], in_=mask_d)
            nmask_sb = const.tile([128, BPC], f32)
            nc.sync.dma_start(out=nmask_sb[:], in_=nmask_d)
            hs_sb = const.tile([N, BPC, D], bf16)
            nc.gpsimd.dma_start(out=hs_sb[:], in_=hs_d)
            ident = const.tile([128, 128], f32)
            make_identity(nc, ident[:])
            ones120 = const.tile([1, BPC * N], bf16)
            nc.vector.memset(ones120[:], 1.0)

            VT_sb = const.tile([128, BPC, NC, 128], bf16)
            ST_dup = const.tile([128, NC, BPC, N, 2], bf16)

            # ---- helpers -------------------------------------------------
            def proj_S(mc):
                # S chunk: ST'[d, (b, n)] for all batches
                ps_s = psp.tile([128, BPC * N], f32, tag="ps")
                for kc in range(NC):
                    nc.tensor.matmul(
                        ps_s[:],
                        WS_sb[:, kc, mc, :],
                        hsT_sb[:, kc, :],
                        start=(kc == 0),
                        stop=False,
                    )
                # + (b_S + b_V) broadcast along (b, n): rank-1 K=1 matmul
                nc.tensor.matmul(
                    ps_s[:],
                    bSV_sb[0:1, mc * 128 : (mc + 1) * 128],
                    ones120[0:1, :],
                    start=False,
                    stop=True,
                )
                # ST_dup[d, mc, b, n, 2] <- ps_s duplicated over pair axis;
                # for chunk 0 split batch 0 out so its first granule (the
                # head of the ACT chain) isn't gated on the full cast
                if mc == 0:
                    nc.vector.tensor_copy(
                        ST_dup[:, mc, 0, :, :],
                        ps_s[:, 0:N].unsqueeze(2).broadcast_to([128, N, 2]),
                    )
                    nc.vector.tensor_copy(
                        ST_dup[:, mc, 1:, :, :].rearrange(
                            "p b n two -> p (b n) two"
                        ),
                        ps_s[:, N:]
                        .unsqueeze(2)
                        .broadcast_to([128, (BPC - 1) * N, 2]),
                    )
                else:
                    nc.vector.tensor_copy(
                        ST_dup[:, mc, :, :, :].rearrange("p b n two -> p (b n) two"),
                        ps_s[:].unsqueeze(2).broadcast_to([128, BPC * N, 2]),
                    )

            def proj_V(mc, b0, b1):
                # V chunk for batches [b0, b1): batches packed in the rhs
                # free dim -> single psum accumulation group (interleaved
                # open groups in one bank corrupt each other's partials)
                nb = b1 - b0
                pv_t = pvp.tile([128, BPC, 128], f32, tag="pv")
                for kc in range(NC):
                    nc.tensor.matmul(
                        pv_t[:, b0:b1, :],
                        WV_sb[:, kc, mc, :],
                        hvT_sb[:, b0:b1, kc, :],
                        start=(kc == 0),
                        stop=(kc == NC - 1),
                    )
                nc.vector.tensor_copy(
                    VT_sb[:, b0:b1, mc, :], pv_t[:, b0:b1, :]
                )

            def ep_add(b, c, ep_slice):
                """e_pre = VT (+bcast over n) + ST' (+bcast over t-pairs)
                for one (batch, chunk) into ep_slice [128, N, 128]."""
                if use_pair:
                    nc.vector.tensor_add(
                        ep_slice.rearrange("p n (t two) -> p n t two", two=2),
                        VT_sb[:, b, c, :]
                        .rearrange("p (t two) -> p t two", two=2)
                        .unsqueeze(1)
                        .broadcast_to([128, N, 64, 2]),
                        ST_dup[:, c, b, :, :]
                        .unsqueeze(2)
                        .broadcast_to([128, N, 64, 2]),
                    )
                else:
                    nc.vector.tensor_add(
                        ep_slice,
                        VT_sb[:, b, c, :].unsqueeze(1).broadcast_to([128, N, 128]),
                        ST_dup[:, c, b, :, 0:1].broadcast_to([128, N, 128]),
                    )

            def beta_mms(b, c, eb, beta_big, n0=0, n1=N):
                for n in range(n0, n1):
                    nc.tensor.matmul(
                        beta_big[:, c, n : n + 1],
                        eb[:, c, n, :],
                        Ww_sb[:, c : c + 1],
                        start=True,
                        stop=True,
                    )

            def granule(b, c, eb, beta_big):
                """1-chunk granule: add (DVE 2x) -> tanh (ACT) -> beta (PE)."""
                ep = eprep.tile([128, N, 128], bf16, tag="ep")
                ep_add(b, c, ep[:])
                nc.scalar.activation(
                    eb[:, c, :, :], ep[:], mybir.ActivationFunctionType.Tanh
                )
                beta_mms(b, c, eb, beta_big)

            def granule2(b, c0, eb, beta_big):
                """2-chunk granule: two adds, one tanh (halves the per-op
                ACT access overhead), beta partials for both chunks."""
                ep = eprep2.tile([128, 2, N, 128], bf16, tag="ep2")
                ep_add(b, c0, ep[:, 0, :, :])
                ep_add(b, c0 + 1, ep[:, 1, :, :])
                nc.scalar.activation(
                    eb[:, c0 : c0 + 2, :, :],
                    ep[:],
                    mybir.ActivationFunctionType.Tanh,
                )
                beta_mms(b, c0, eb, beta_big)
                beta_mms(b, c0 + 1, eb, beta_big)

            def granule_tail(b, c, eb, beta_big):
                """Last granule: tanh split by n-halves so the final beta
                matmuls and softmax start earlier."""
                ep = eprep.tile([128, N, 128], bf16, tag="ep")
                ep_add(b, c, ep[:])
                h = N // 2
                nc.scalar.activation(
                    eb[:, c, 0:h, :],
                    ep[:, 0:h, :],
                    mybir.ActivationFunctionType.Tanh,
                )
                beta_mms(b, c, eb, beta_big, 0, h)
                nc.scalar.activation(
                    eb[:, c, h:N, :],
                    ep[:, h:N, :],
                    mybir.ActivationFunctionType.Tanh,
                )
                beta_mms(b, c, eb, beta_big, h, N)

            def granule_head(b, c, eb, beta_big):
                """First granule: add and tanh split by n-halves so the ACT
                chain starts on a half-size dependency."""
                h = N // 2
                ep = eprep.tile([128, N, 128], bf16, tag="ep")
                for n0, n1 in ((0, h), (h, N)):
                    if use_pair:
                        nc.vector.tensor_add(
                            ep[:, n0:n1, :].rearrange(
                                "p n (t two) -> p n t two", two=2
                            ),
                            VT_sb[:, b, c, :]
                            .rearrange("p (t two) -> p t two", two=2)
                            .unsqueeze(1)
                            .broadcast_to([128, h, 64, 2]),
                            ST_dup[:, c, b, n0:n1, :]
                            .unsqueeze(2)
                            .broadcast_to([128, h, 64, 2]),
                        )
                    else:
                        nc.vector.tensor_add(
                            ep[:, n0:n1, :],
                            VT_sb[:, b, c, :]
                            .unsqueeze(1)
                            .broadcast_to([128, h, 128]),
                            ST_dup[:, c, b, n0:n1, 0:1].broadcast_to(
                                [128, h, 128]
                            ),
                        )
                    nc.scalar.activation(
                        eb[:, c, n0:n1, :],
                        ep[:, n0:n1, :],
                        mybir.ActivationFunctionType.Tanh,
                    )
                    beta_mms(b, c, eb, beta_big, n0, n1)

            def softmax_final(b, beta_big):
                # ---- masked softmax (faithful to reference) ----
                m_b = mask_sb[:, b, :]
                bcp = softp.tile([128, NC, N], f32, tag="bcp")
                nc.vector.tensor_copy(bcp[:], beta_big[:])
                s2 = softp.tile([128, 2, N], f32, tag="s2")
                nc.vector.tensor_add(s2[:], bcp[:, 0:2, :], bcp[:, 2:4, :])
                qa = softp.tile([128, N], f32, tag="qa")
                # qa = (s01 + b_w) + s23
                nc.vector.scalar_tensor_tensor(
                    qa[:],
                    s2[:, 0, :],
                    bw_sb[:],
                    s2[:, 1, :],
                    op0=mybir.AluOpType.add,
                    op1=mybir.AluOpType.add,
                )
                # exp directly on qa: masked positions are killed in the
                # numerator by the zeroed h_s rows, and the denominator
                # branch (mul/reduce/recip) runs parallel to the
                # transpose->matmul chain instead of serializing before exp
                t1 = softp.tile([128, N], f32, tag="t1")
                nc.scalar.activation(t1[:], qa[:], mybir.ActivationFunctionType.Exp)
                q = softp.tile([128, N], f32, tag="q1")
                nc.vector.tensor_mul(q[:], t1[:], m_b)
                Qs = softp.tile([128, 1], f32, tag="Z1")
                nc.vector.tensor_reduce(
                    Qs[:], q[:], mybir.AxisListType.X, mybir.AluOpType.add
                )
                recip = softp.tile([128, 1], f32, tag="recip")
                nc.vector.reciprocal(recip[:], Qs[:])
                if debug:
                    # hijack out[b]: pack debug views into spare columns
                    dbgt = outp.tile([128, D], f32, tag="dbgt")
                    nc.vector.memset(dbgt[:], 0.0)
                    nc.vector.tensor_copy(dbgt[:, 0:N], qa[:])
                    nc.vector.tensor_copy(dbgt[:, 32 : 32 + N], t1[:])
                    nc.sync.dma_start(out=out_d[b], in_=dbgt[:])
                    return
                # ---- out[b] = (t1 @ h_s_masked[b]) * recip ----
                qT_ps = pqtp.tile([N, 128], f32, tag="qt")
                nc.tensor.transpose(qT_ps[:], t1[:], ident[:])
                qT = softp.tile([N, 128], bf16, tag="qTs")
                nc.vector.tensor_copy(qT[:], qT_ps[:])
                out_ps = pfinp.tile([128, D], f32, tag="out")
                nc.tensor.matmul(
                    out_ps[:], qT[:], hs_sb[:, b, :], start=True, stop=True
                )
                out_sb = outp.tile([128, D], out_dt, tag="osb")
                if b == BPC - 1:
                    # tail: ACT is idle by now and sits closer to PSUM; split
                    # by column halves so the DMA overlaps the second copy
                    for hf in range(2):
                        cs2 = slice(hf * (D // 2), (hf + 1) * (D // 2))
                        nc.scalar.activation(
                            out_sb[:, cs2],
                            out_ps[:, cs2],
                            mybir.ActivationFunctionType.Copy,
                            scale=recip[:],
                        )
                        nc.sync.dma_start(
                            out=out_d[b][:, cs2], in_=out_sb[:, cs2]
                        )
                else:
                    nc.vector.tensor_scalar_mul(out_sb[:], out_ps[:], recip[:])
                    nc.sync.dma_start(out=out_d[b], in_=out_sb[:])

            # ---- projections interleaved with batch-0 granules so the
            # ACT tanh chain starts as early as possible; chunk 0's V
            # projection is split so batch 0's first granule isn't gated
            # on the other batches' V work ----
            batch_tiles = {}
            for mc in range(NC):
                proj_S(mc)
                if mc == 0:
                    proj_V(0, 0, 1)
                    eb0 = ebigp.tile([128, NC, N, 128], bf16, tag="e")
                    bb0 = pbetap.tile([128, NC, N], f32, tag="beta")
                    batch_tiles[0] = (eb0, bb0)
                    granule_head(0, mc, *batch_tiles[0])
                else:
                    proj_V(mc, 0, BPC)
                    granule(0, mc, *batch_tiles[0])
            proj_V(0, 1, BPC)

            # ---- remaining batches, softmax/final pipelined one batch late;
            # middle batches use merged 2-chunk tanh ops, the very last
            # granule splits its tanh so the tail chain starts early ----
            for b in range(1, BPC):
                eb = ebigp.tile([128, NC, N, 128], bf16, tag="e")
                beta_big = pbetap.tile([128, NC, N], f32, tag="beta")
                batch_tiles[b] = (eb, beta_big)
                # previous batch's softmax emitted mid-batch so its DVE ops
                # don't bunch up ahead of this batch's later adds
                if b < BPC - 1:
                    granule2(b, 0, eb, beta_big)
                    softmax_final(b - 1, batch_tiles[b - 1][1])
                    granule2(b, 2, eb, beta_big)
                else:
                    # last batch: softmax first so its DVE ops don't sit
                    # between this batch's adds (ACT slack is at the start
                    # of this section, the end is the critical tail); split
                    # the last two granules' tanh by n-halves for finer
                    # chain-end dependencies
                    softmax_final(b - 1, batch_tiles[b - 1][1])
                    granule2(b, 0, eb, beta_big)
                    granule_tail(b, 2, eb, beta_big)
                    granule_tail(b, 3, eb, beta_big)
            softmax_final(BPC - 1, batch_tiles[BPC - 1][1])

    nc.compile()
    return nc


def _get_nc():
    variant = os.environ.get("KERNEL_VARIANT", "bf16pair")
    if variant not in _CACHE:
        _CACHE[variant] = _build(variant)
    return _CACHE[variant]


def _make_in_maps(variant, h_s, h_v, lengths, W_S, b_S, W_V, b_V, W_w, b_w):
    f32 = np.float32
    h_s = np.asarray(h_s, dtype=f32)
    h_v = np.asarray(h_v, dtype=f32)
    mask = (
        np.asarray(lengths).reshape(B, 1) >= np.arange(1, N + 1).reshape(1, N)
    ).astype(f32)
    # weights, chunked + cast once (shared across cores)
    WS = np.ascontiguousarray(
        np.asarray(W_S, f32).reshape(NC, 128, NC, 128).transpose(1, 0, 2, 3)
    ).astype(np.float32)  # keep f32 here; cast below via bf16 view helper
    WV = np.ascontiguousarray(
        np.asarray(W_V, f32).reshape(NC, 128, NC, 128).transpose(1, 0, 2, 3)
    )
    Ww = np.ascontiguousarray(np.asarray(W_w, f32).reshape(NC, 128).T)
    bSV = (np.asarray(b_S, f32) + np.asarray(b_V, f32)).reshape(1, D)
    bw_rep = np.full((128, 1), f32(np.asarray(b_w).reshape(-1)[0]), dtype=f32)

    try:
        import ml_dtypes

        bf16 = ml_dtypes.bfloat16
    except ImportError:  # numpy >= 2.3 may lack ml_dtypes; fall back via jax
        import jax.numpy as jnp

        bf16 = jnp.bfloat16

    def to_bf16(x):
        if variant == "f32plain":
            return np.ascontiguousarray(x, dtype=np.float32)
        return np.asarray(x, dtype=bf16)

    WS_b = to_bf16(WS)
    WV_b = to_bf16(WV)
    Ww_b = to_bf16(Ww)
    bSV_b = to_bf16(bSV)

    in_maps = []
    for core in range(NCORES):
        sl = slice(core * BPC, (core + 1) * BPC)
        hv_c = h_v[sl]  # (BPC, T, D)
        hs_c = h_s[sl]  # (BPC, N, D)
        hvT = np.ascontiguousarray(
            hv_c.reshape(BPC, T, NC, 128).transpose(3, 0, 2, 1)
        )  # (128p, b, kc, t)
        hsT = np.ascontiguousarray(
            hs_c.reshape(BPC, N, NC, 128).transpose(3, 2, 0, 1)
        ).reshape(128, NC, BPC * N)  # (128p, kc, (b n))
        # masked rows of h_s zeroed: lets t1 feed the final matmul unmasked
        hs_r = to_bf16(
            np.ascontiguousarray(
                hs_c.transpose(1, 0, 2) * mask[sl].T[:, :, None]
            )
        )  # (N, b, D), bf16: final einsum runs bf16 (FWL on qT)
        mask_bc = np.ascontiguousarray(
            np.broadcast_to(mask[sl][None, :, :], (128, BPC, N)), dtype=f32
        )
        nmask = np.ascontiguousarray(
            np.broadcast_to(
                (N - np.asarray(lengths)[sl].astype(f32))[None, :], (128, BPC)
            )
        )
        in_maps.append(
            {
                "hvT": to_bf16(hvT),
                "hsT": to_bf16(hsT),
                "hs": hs_r,
                "WS": WS_b,
                "WV": WV_b,
                "Ww": Ww_b,
                "bSV": bSV_b,
                "bw": bw_rep,
                "mask": mask_bc,
                "nmask": nmask,
            }
        )
    return in_maps


def run(inputs: dict, trace: bool = False):
    """Run on 8 NeuronCores; returns (output, BassKernelResults)."""
    from concourse import bass_utils

    nc = _get_nc()
    variant = os.environ.get("KERNEL_VARIANT", "bf16pair")
    in_maps = _make_in_maps(variant, **inputs)
    res = bass_utils.run_bass_kernel_spmd(
        nc, in_maps, core_ids=list(range(NCORES)), trace=trace
    )
    outs = [r["out"] for r in res.results]
    full = np.concatenate(outs, axis=0).astype(np.float32)
    return full, res


def kernel(**inputs) -> np.ndarray:
    out, _ = run(inputs, trace=False)
    return out



# revision 5
# speedup vs baseline: 1.3777x; 1.3777x over previous
"""Trainium2 Bass kernel for InteractorwoLSTM additive attention.

out[b,t,:] = alpha[b,t,:] @ h_s[b]  with
  beta[b,t,n] = W_w . tanh(h_s[b,n]@W_S + b_S + h_v[b,t]@W_V + b_V) + b_w
  alpha = masked-softmax(beta) per reference semantics.

v3 design: length-truncated slots.

Positions n >= lengths[b] never influence the output (the reference's
masked-softmax renormalization cancels them), so the tanh/add/beta work
for those positions can be skipped entirely. lengths are known on the
host before compile, so the kernel program is built for the actual
length profile:

  - batches sorted by length desc; slot k on core c holds sorted rank
    8k+c. SPMD shares one program across cores, so slot k's n-bound is
    max over cores = L(rank 8k). For seed-0 data: [30,17,12,8] -> 67
    packed n-columns instead of 4*30=120 (~44% less ACT/DVE/PE work;
    ACT tanh is the bottleneck engine at 1 elem/cycle/lane).
  - per core: S-side tensors packed over (slot, n<bound) columns; the
    pad rows (n in [L_b, bound)) carry zeroed h_s so their garbage
    scores are killed by the mask mult + zeroed final-matmul rows.
  - everything else as v2: host-side transposes, weight-stationary bf16
    projections with (b_S+b_V) folded via rank-1 matmul, DVE 2x-pair
    broadcast add, ACT tanh, per-n beta matmuls into PSUM, softmax with
    exp (no max-sub needed; |beta| <= sum|W_w|+|b_w| keeps exp finite),
    denominator folded into the output copy.
  - a dummy tanh at the top of the program forces the ACT table load
    (~2.7us) to overlap the initial weight DMAs.
"""

import numpy as np

B, T, N = 32, 128, 30
D = 512
NCORES = 8
BPC = B // NCORES  # batch slots per core
NC = D // 128  # 4 chunks of 128 along D

_CACHE = {}


def _build(bounds):
    import concourse.bacc as bacc
    import concourse.tile as tile
    from concourse import mybir
    from concourse.masks import make_identity

    f32 = mybir.dt.float32
    bf16 = mybir.dt.bfloat16

    offs = [0]
    for b in bounds:
        offs.append(offs[-1] + b)
    P = offs[-1]
    B0 = bounds[0]

    nc = bacc.Bacc(
        "TRN2",
        target_bir_lowering=False,
        debug=False,
        enable_asserts=True,
        num_devices=NCORES,
    )

    # ---- DRAM I/O (all host-prepped layouts) ----
    hvT_d = nc.dram_tensor("hvT", [128, BPC, NC, 128], bf16, kind="ExternalInput").ap()
    hsT_d = nc.dram_tensor("hsT", [128, NC, P], bf16, kind="ExternalInput").ap()
    hs_d = nc.dram_tensor("hs", [B0, BPC, D], bf16, kind="ExternalInput").ap()
    WS_d = nc.dram_tensor("WS", [128, NC, NC, 128], bf16, kind="ExternalInput").ap()
    WV_d = nc.dram_tensor("WV", [128, NC, NC, 128], bf16, kind="ExternalInput").ap()
    Ww_d = nc.dram_tensor("Ww", [128, NC], bf16, kind="ExternalInput").ap()
    bSV_d = nc.dram_tensor("bSV", [1, D], bf16, kind="ExternalInput").ap()
    bw_d = nc.dram_tensor("bw", [128, 1], f32, kind="ExternalInput").ap()
    mask_d = nc.dram_tensor("mask", [128, P], f32, kind="ExternalInput").ap()
    out_d = nc.dram_tensor("out", [BPC, T, D], bf16, kind="ExternalOutput").ap()

    with tile.TileContext(nc) as tc:
        with (
            tc.tile_pool(name="const", bufs=1) as const,
            tc.tile_pool(name="epre", bufs=3) as eprep,
            tc.tile_pool(name="epre2", bufs=3) as eprep2,
            tc.tile_pool(name="ebig", bufs=1) as ebigp,
            tc.tile_pool(name="soft", bufs=3) as softp,
            tc.tile_pool(name="outp", bufs=3) as outp,
            tc.tile_pool(name="pv", bufs=2, space="PSUM") as pvp,
            tc.tile_pool(name="ps", bufs=2, space="PSUM") as psp,
            tc.tile_pool(name="pbeta", bufs=2, space="PSUM") as pbetap,
            tc.tile_pool(name="pqt", bufs=1, space="PSUM") as pqtp,
            tc.tile_pool(name="pfin", bufs=1, space="PSUM") as pfinp,
        ):
            # ---- force the exp/tanh ACT table load to start at t=0 so it
            # hides under the weight DMAs (the set covers tanh+exp+copy)
            warm = const.tile([1, 2], f32)
            nc.vector.memset(warm[:], 0.0)
            warm2 = const.tile([1, 2], f32)
            nc.scalar.activation(warm2[:], warm[:], mybir.ActivationFunctionType.Tanh)

            # ---- constants / weights; DMAs split so the chunk-0 pieces the
            # first granule needs arrive first, spread across queues ----
            WS_sb = const.tile([128, NC, NC, 128], bf16)
            nc.sync.dma_start(out=WS_sb[:, :, 0, :], in_=WS_d[:, :, 0, :])
            hsT_sb = const.tile([128, NC, P], bf16)
            nc.scalar.dma_start(out=hsT_sb[:], in_=hsT_d)
            bSV_sb = const.tile([1, D], bf16)
            nc.scalar.dma_start(out=bSV_sb[:], in_=bSV_d)
            WV_sb = const.tile([128, NC, NC, 128], bf16)
            nc.gpsimd.dma_start(out=WV_sb[:, :, 0, :], in_=WV_d[:, :, 0, :])
            hvT_sb = const.tile([128, BPC, NC, 128], bf16)
            nc.scalar.dma_start(out=hvT_sb[:, 0, :, :], in_=hvT_d[:, 0, :, :])
            nc.sync.dma_start(out=WS_sb[:, :, 1:, :], in_=WS_d[:, :, 1:, :])
            nc.gpsimd.dma_start(out=WV_sb[:, :, 1:, :], in_=WV_d[:, :, 1:, :])
            nc.scalar.dma_start(out=hvT_sb[:, 1:, :, :], in_=hvT_d[:, 1:, :, :])
            Ww_sb = const.tile([128, NC], bf16)
            nc.sync.dma_start(out=Ww_sb[:], in_=Ww_d)
            bw_sb = const.tile([128, 1], f32)
            nc.sync.dma_start(out=bw_sb[:], in_=bw_d)
            mask_sb = const.tile([128, P], f32)
            nc.gpsimd.dma_start(out=mask_sb[:], in_=mask_d)
            hs_sb = const.tile([B0, BPC, D], bf16)
            nc.gpsimd.dma_start(out=hs_sb[:], in_=hs_d)
            ident = const.tile([128, 128], f32)
            make_identity(nc, ident[:])
            onesP = const.tile([1, P], bf16)
            nc.vector.memset(onesP[:], 1.0)

            VT_sb = const.tile([128, BPC, NC, 128], bf16)
            ST_dup = const.tile([128, NC, P, 2], bf16)

            # ---- helpers -------------------------------------------------
            def proj_S(mc):
                # S chunk: ST'[d, packed(slot, n)] for all slots
                ps_s = psp.tile([128, P], f32, tag="ps")
                for kc in range(NC):
                    nc.tensor.matmul(
                        ps_s[:],
                        WS_sb[:, kc, mc, :],
                        hsT_sb[:, kc, :],
                        start=(kc == 0),
                        stop=False,
                    )
                # + (b_S + b_V) broadcast along packed cols: rank-1 K=1 matmul
                nc.tensor.matmul(
                    ps_s[:],
                    bSV_sb[0:1, mc * 128 : (mc + 1) * 128],
                    onesP[0:1, :],
                    start=False,
                    stop=True,
                )
                # ST_dup[d, mc, p, 2] <- ps_s duplicated over pair axis; for
                # chunk 0 split slot 0 out so the head of the ACT chain isn't
                # gated on the full-P cast
                if mc == 0:
                    nc.vector.tensor_copy(
                        ST_dup[:, mc, 0 : offs[1], :],
                        ps_s[:, 0 : offs[1]]
                        .unsqueeze(2)
                        .broadcast_to([128, bounds[0], 2]),
                    )
                    nc.vector.tensor_copy(
                        ST_dup[:, mc, offs[1] :, :],
                        ps_s[:, offs[1] :]
                        .unsqueeze(2)
                        .broadcast_to([128, P - offs[1], 2]),
                    )
                else:
                    nc.vector.tensor_copy(
                        ST_dup[:, mc, :, :],
                        ps_s[:].unsqueeze(2).broadcast_to([128, P, 2]),
                    )

            def proj_V(mc, k0, k1):
                # V chunk for slots [k0, k1): slots packed in the rhs free
                # dim -> single psum accumulation group
                pv_t = pvp.tile([128, BPC, 128], f32, tag="pv")
                for kc in range(NC):
                    nc.tensor.matmul(
                        pv_t[:, k0:k1, :],
                        WV_sb[:, kc, mc, :],
                        hvT_sb[:, k0:k1, kc, :],
                        start=(kc == 0),
                        stop=(kc == NC - 1),
                    )
                nc.vector.tensor_copy(VT_sb[:, k0:k1, mc, :], pv_t[:, k0:k1, :])

            def ep_add(k, c, ep_slice, n0, n1):
                """e_pre = VT (+bcast over n) + ST' (+bcast over t-pairs)
                for slot k chunk c, rows [n0,n1), into ep_slice."""
                nn = n1 - n0
                nc.vector.tensor_add(
                    ep_slice.rearrange("p n (t two) -> p n t two", two=2),
                    VT_sb[:, k, c, :]
                    .rearrange("p (t two) -> p t two", two=2)
                    .unsqueeze(1)
                    .broadcast_to([128, nn, 64, 2]),
                    ST_dup[:, c, offs[k] + n0 : offs[k] + n1, :]
                    .unsqueeze(2)
                    .broadcast_to([128, nn, 64, 2]),
                )

            def beta_mms(k, c, eb, beta_big, n0, n1):
                for n in range(n0, n1):
                    nc.tensor.matmul(
                        beta_big[:, c, n : n + 1],
                        eb[:, c, n, :],
                        Ww_sb[:, c : c + 1],
                        start=True,
                        stop=True,
                    )

            def granule(k, c, eb, beta_big):
                """1-chunk granule: add (DVE 2x) -> tanh (ACT) -> beta (PE)."""
                bk = bounds[k]
                ep = eprep.tile([128, B0, 128], bf16, tag="ep")
                ep_add(k, c, ep[:, 0:bk, :], 0, bk)
                nc.scalar.activation(
                    eb[:, c, :, :], ep[:, 0:bk, :], mybir.ActivationFunctionType.Tanh
                )
                beta_mms(k, c, eb, beta_big, 0, bk)

            def granule2(k, c0, eb, beta_big):
                """2-chunk granule: two adds, one tanh (halves the per-op
                ACT access overhead), beta partials for both chunks."""
                bk = bounds[k]
                ep = eprep2.tile([128, 2, bounds[1], 128], bf16, tag="ep2")
                ep_add(k, c0, ep[:, 0, 0:bk, :], 0, bk)
                ep_add(k, c0 + 1, ep[:, 1, 0:bk, :], 0, bk)
                nc.scalar.activation(
                    eb[:, c0 : c0 + 2, :, :],
                    ep[:, :, 0:bk, :],
                    mybir.ActivationFunctionType.Tanh,
                )
                beta_mms(k, c0, eb, beta_big, 0, bk)
                beta_mms(k, c0 + 1, eb, beta_big, 0, bk)

            def granule_head(k, c, eb, beta_big):
                """First granule: add and tanh split by n-halves so the ACT
                chain starts on a half-size dependency."""
                bk = bounds[k]
                h = max(1, bk // 2)
                ep = eprep.tile([128, B0, 128], bf16, tag="ep")
                for n0, n1 in ((0, h), (h, bk)):
                    if n1 <= n0:
                        continue
                    ep_add(k, c, ep[:, n0:n1, :], n0, n1)
                    nc.scalar.activation(
                        eb[:, c, n0:n1, :],
                        ep[:, n0:n1, :],
                        mybir.ActivationFunctionType.Tanh,
                    )
                    beta_mms(k, c, eb, beta_big, n0, n1)

            def softmax_final(k, beta_big):
                bk = bounds[k]
                # ---- masked softmax (faithful to reference) ----
                bcp = softp.tile([128, NC, B0], f32, tag="bcp")
                nc.vector.tensor_copy(bcp[:, :, 0:bk], beta_big[:])
                s2 = softp.tile([128, 2, B0], f32, tag="s2")
                nc.vector.tensor_add(
                    s2[:, :, 0:bk], bcp[:, 0:2, 0:bk], bcp[:, 2:4, 0:bk]
                )
                qa = softp.tile([128, B0], f32, tag="qa")
                # qa = (s01 + b_w) + s23
                nc.vector.scalar_tensor_tensor(
                    qa[:, 0:bk],
                    s2[:, 0, 0:bk],
                    bw_sb[:],
                    s2[:, 1, 0:bk],
                    op0=mybir.AluOpType.add,
                    op1=mybir.AluOpType.add,
                )
                # exp directly on qa: pad positions are killed in the
                # numerator by the zeroed h_s rows and in the denominator by
                # the mask mult
                t1 = softp.tile([128, B0], f32, tag="t1")
                nc.scalar.activation(
                    t1[:, 0:bk], qa[:, 0:bk], mybir.ActivationFunctionType.Exp
                )
                q = softp.tile([128, B0], f32, tag="q1")
                nc.vector.tensor_mul(
                    q[:, 0:bk], t1[:, 0:bk], mask_sb[:, offs[k] : offs[k] + bk]
                )
                Qs = softp.tile([128, 1], f32, tag="Z1")
                nc.vector.tensor_reduce(
                    Qs[:], q[:, 0:bk], mybir.AxisListType.X, mybir.AluOpType.add
                )
                recip = softp.tile([128, 1], f32, tag="recip")
                nc.vector.reciprocal(recip[:], Qs[:])
                # ---- out[k] = (t1 @ h_s_masked[k]) * recip ----
                qT_ps = pqtp.tile([B0, 128], f32, tag="qt")
                nc.tensor.transpose(qT_ps[0:bk, :], t1[:, 0:bk], ident[:])
                qT = softp.tile([B0, 128], bf16, tag="qTs")
                nc.vector.tensor_copy(qT[0:bk, :], qT_ps[0:bk, :])
                out_ps = pfinp.tile([128, D], f32, tag="out")
                nc.tensor.matmul(
                    out_ps[:], qT[0:bk, :], hs_sb[0:bk, k, :], start=True, stop=True
                )
                out_sb = outp.tile([128, D], bf16, tag="osb")
                if k == BPC - 1:
                    # tail: ACT is idle by now and sits closer to PSUM; split
                    # by column halves so the DMA overlaps the second copy
                    for hf in range(2):
                        cs2 = slice(hf * (D // 2), (hf + 1) * (D // 2))
                        nc.scalar.activation(
                            out_sb[:, cs2],
                            out_ps[:, cs2],
                            mybir.ActivationFunctionType.Copy,
                            scale=recip[:],
                        )
                        nc.sync.dma_start(out=out_d[k][:, cs2], in_=out_sb[:, cs2])
                else:
                    nc.vector.tensor_scalar_mul(out_sb[:], out_ps[:], recip[:])
                    nc.sync.dma_start(out=out_d[k], in_=out_sb[:])

            # ---- projections interleaved with slot-0 granules so the ACT
            # tanh chain starts as early as possible; chunk 0's V projection
            # is split so slot 0's first granule isn't gated on the other
            # slots' V work ----
            slot_tiles = {}

            def alloc_slot(k):
                slot_tiles[k] = (
                    ebigp.tile(
                        [128, NC, bounds[k], 128], bf16, tag=f"e{k}", name=f"eb{k}"
                    ),
                    pbetap.tile(
                        [128, NC, bounds[k]], f32, tag="beta", name=f"bb{k}"
                    ),
                )

            for mc in range(NC):
                proj_S(mc)
                if mc == 0:
                    proj_V(0, 0, 1)
                    alloc_slot(0)
                    granule_head(0, mc, *slot_tiles[0])
                else:
                    proj_V(mc, 0, BPC)
                    granule(0, mc, *slot_tiles[0])
            proj_V(0, 1, BPC)

            # ---- remaining slots, softmax/final pipelined one slot late ----
            for k in range(1, BPC):
                alloc_slot(k)
                eb, beta_big = slot_tiles[k]
                if k < BPC - 1:
                    granule2(k, 0, eb, beta_big)
                    softmax_final(k - 1, slot_tiles[k - 1][1])
                    granule2(k, 2, eb, beta_big)
                else:
                    # last slot: softmax first so its DVE ops don't sit
                    # between this slot's adds (the end is the critical tail)
                    softmax_final(k - 1, slot_tiles[k - 1][1])
                    granule2(k, 0, eb, beta_big)
                    granule2(k, 2, eb, beta_big)
            softmax_final(BPC - 1, slot_tiles[BPC - 1][1])

    nc.compile()
    return nc


def _get_nc(bounds):
    key = tuple(bounds)
    if key not in _CACHE:
        _CACHE[key] = _build(list(bounds))
    return _CACHE[key]


def _plan(lengths):
    """Sort batches by length desc; slot k on core c <- sorted rank 8k+c.
    Returns (order, bounds)."""
    lengths = np.asarray(lengths).reshape(-1)
    order = np.argsort(-lengths, kind="stable")
    bounds = [int(lengths[order[NCORES * k]]) for k in range(BPC)]
    return order, bounds


def _make_in_maps(order, bounds, h_s, h_v, lengths, W_S, b_S, W_V, b_V, W_w, b_w):
    f32 = np.float32
    h_s = np.asarray(h_s, dtype=f32)
    h_v = np.asarray(h_v, dtype=f32)
    lengths = np.asarray(lengths).reshape(-1)
    offs = np.concatenate([[0], np.cumsum(bounds)]).astype(int)
    P = int(offs[-1])
    B0 = bounds[0]

    # weights, chunked + cast once (shared across cores)
    WS = np.ascontiguousarray(
        np.asarray(W_S, f32).reshape(NC, 128, NC, 128).transpose(1, 0, 2, 3)
    )
    WV = np.ascontiguousarray(
        np.asarray(W_V, f32).reshape(NC, 128, NC, 128).transpose(1, 0, 2, 3)
    )
    Ww = np.ascontiguousarray(np.asarray(W_w, f32).reshape(NC, 128).T)
    bSV = (np.asarray(b_S, f32) + np.asarray(b_V, f32)).reshape(1, D)
    bw_rep = np.full((128, 1), f32(np.asarray(b_w).reshape(-1)[0]), dtype=f32)

    try:
        import ml_dtypes

        bf16 = ml_dtypes.bfloat16
    except ImportError:
        import jax.numpy as jnp

        bf16 = jnp.bfloat16

    def to_bf16(x):
        return np.asarray(x, dtype=bf16)

    WS_b = to_bf16(WS)
    WV_b = to_bf16(WV)
    Ww_b = to_bf16(Ww)
    bSV_b = to_bf16(bSV)

    in_maps = []
    for core in range(NCORES):
        batches = [int(order[NCORES * k + core]) for k in range(BPC)]
        hv_c = h_v[batches]  # (BPC, T, D)
        hvT = np.ascontiguousarray(
            hv_c.reshape(BPC, T, NC, 128).transpose(3, 0, 2, 1)
        )  # (128p, slot, kc, t)
        hsT = np.zeros((128, NC, P), dtype=f32)
        hs_r = np.zeros((B0, BPC, D), dtype=f32)  # (n, slot, D), masked rows 0
        mask_bc = np.zeros((128, P), dtype=f32)
        for k, b in enumerate(batches):
            L = int(lengths[b])
            bk = bounds[k]
            Lk = min(L, bk)
            hk = h_s[b, :Lk]  # (Lk, D)
            hsT[:, :, offs[k] : offs[k] + Lk] = hk.reshape(Lk, NC, 128).transpose(
                2, 1, 0
            )
            hs_r[:Lk, k, :] = hk
            mask_bc[:, offs[k] : offs[k] + Lk] = 1.0
        in_maps.append(
            {
                "hvT": to_bf16(hvT),
                "hsT": to_bf16(hsT),
                "hs": to_bf16(hs_r),
                "WS": WS_b,
                "WV": WV_b,
                "Ww": Ww_b,
                "bSV": bSV_b,
                "bw": bw_rep,
                "mask": mask_bc,
            }
        )
    return in_maps


def run(inputs: dict, trace: bool = False):
    """Run on 8 NeuronCores; returns (output, BassKernelResults)."""
    from concourse import bass_utils

    order, bounds = _plan(inputs["lengths"])
    nc = _get_nc(bounds)
    in_maps = _make_in_maps(order, bounds, **inputs)
    res = bass_utils.run_bass_kernel_spmd(
        nc, in_maps, core_ids=list(range(NCORES)), trace=trace
    )
    full = np.zeros((B, T, D), dtype=np.float32)
    for core in range(NCORES):
        o = np.asarray(res.results[core]["out"], dtype=np.float32)
        for k in range(BPC):
            full[int(order[NCORES * k + core])] = o[k]
    return full, res


def kernel(**inputs) -> np.ndarray:
    out, _ = run(inputs, trace=False)
    return out


# revision 6
# speedup vs baseline: 1.3790x; 1.0009x over previous
"""Trainium2 Bass kernel for InteractorwoLSTM additive attention.

out[b,t,:] = alpha[b,t,:] @ h_s[b]  with
  beta[b,t,n] = W_w . tanh(h_s[b,n]@W_S + b_S + h_v[b,t]@W_V + b_V) + b_w
  alpha = masked-softmax(beta) per reference semantics.

v4 design: length-truncated slots + JIT DMA + tight ramp/tail.

Positions n >= lengths[b] never influence the output (the reference's
masked-softmax renormalization cancels them), so the tanh/add/beta work
for those positions is skipped. lengths are known on the host before
compile, so the program is built for the actual length profile:
batches sorted by length desc; slot k on core c holds sorted rank
8k+c; slot k's n-bound = L(rank 8k) (SPMD shares one program). For the
seed-0 data: bounds [30,17,12,8] -> 67 packed n-columns vs 120
(~44% less work on ACT -- the bottleneck engine at 1 elem/cycle/lane).

v4 over v3 (v3 = 59.8us, ACT busy 34.5us, ramp-to-first-tanh 16us,
tail 9us):
  - weight/input DRAM layouts are chunk-outermost so each DMA piece is
    a contiguous 1KB-per-partition run; pieces are ordered/queued so
    the first granule's deps (WS0/hsT/WV0/hvT0) land first.
  - slot-0 beta matmuls are emitted one chunk late so the PE queue
    never blocks projections behind tanh-gated work.
  - softmax chunk-sums read beta PSUM in c01/c23 halves as soon as
    each half is done -> the tail only carries the c23 copy.
  - last granule and final matmul/copy split for a shorter tail.
  - dummy tanh at the top forces the ACT table load (~2.7us) under the
    initial DMAs.
"""

import numpy as np

B, T, N = 32, 128, 30
D = 512
NCORES = 8
BPC = B // NCORES  # batch slots per core
NC = D // 128  # 4 chunks of 128 along D

_CACHE = {}


def _build(bounds):
    import concourse.bacc as bacc
    import concourse.tile as tile
    from concourse import mybir
    from concourse.masks import make_identity

    f32 = mybir.dt.float32
    bf16 = mybir.dt.bfloat16

    offs = [0]
    for b in bounds:
        offs.append(offs[-1] + b)
    P = offs[-1]
    B0 = bounds[0]

    nc = bacc.Bacc(
        "TRN2",
        target_bir_lowering=False,
        debug=False,
        enable_asserts=True,
        num_devices=NCORES,
    )

    # ---- DRAM I/O (host-prepped layouts, chunk-outermost for DMA) ----
    hvT_d = nc.dram_tensor("hvT", [BPC, 128, NC, 128], bf16, kind="ExternalInput").ap()
    hsT_d = nc.dram_tensor("hsT", [128, NC, P], bf16, kind="ExternalInput").ap()
    hs_d = nc.dram_tensor("hs", [B0, BPC, D], bf16, kind="ExternalInput").ap()
    WS_d = nc.dram_tensor("WS", [NC, 128, NC, 128], bf16, kind="ExternalInput").ap()
    WV_d = nc.dram_tensor("WV", [NC, 128, NC, 128], bf16, kind="ExternalInput").ap()
    Ww_d = nc.dram_tensor("Ww", [128, NC], bf16, kind="ExternalInput").ap()
    bSV_d = nc.dram_tensor("bSV", [1, D], bf16, kind="ExternalInput").ap()
    bw_d = nc.dram_tensor("bw", [128, 1], f32, kind="ExternalInput").ap()
    mask_d = nc.dram_tensor("mask", [128, P], f32, kind="ExternalInput").ap()
    out_d = nc.dram_tensor("out", [BPC, T, D], bf16, kind="ExternalOutput").ap()

    with tile.TileContext(nc) as tc:
        with (
            tc.tile_pool(name="const", bufs=1) as const,
            tc.tile_pool(name="epre", bufs=3) as eprep,
            tc.tile_pool(name="epre2", bufs=3) as eprep2,
            tc.tile_pool(name="ebig", bufs=1) as ebigp,
            tc.tile_pool(name="soft", bufs=3) as softp,
            tc.tile_pool(name="outp", bufs=3) as outp,
            tc.tile_pool(name="pv", bufs=2, space="PSUM") as pvp,
            tc.tile_pool(name="ps", bufs=2, space="PSUM") as psp,
            tc.tile_pool(name="pbeta", bufs=2, space="PSUM") as pbetap,
            tc.tile_pool(name="pqt", bufs=1, space="PSUM") as pqtp,
            tc.tile_pool(name="pfin", bufs=1, space="PSUM") as pfinp,
        ):
            # ---- force the exp/tanh ACT table load to start at t=0 so it
            # hides under the weight DMAs (the set covers tanh+exp+copy)
            warm = const.tile([1, 2], f32)
            nc.vector.memset(warm[:], 0.0)
            warm2 = const.tile([1, 2], f32)
            nc.scalar.activation(warm2[:], warm[:], mybir.ActivationFunctionType.Tanh)

            # ---- inputs: each queue's pieces ordered by first use ----
            WS_sb = const.tile([128, NC, NC, 128], bf16)  # [p, mc, kc, 128]
            WV_sb = const.tile([128, NC, NC, 128], bf16)  # [p, mc, kc, 128]
            hvT_sb = const.tile([128, BPC, NC, 128], bf16)  # [p, slot, kc, t]
            hsT_sb = const.tile([128, NC, P], bf16)
            bSV_sb = const.tile([1, D], bf16)
            Ww_sb = const.tile([128, NC], bf16)
            bw_sb = const.tile([128, 1], f32)
            mask_sb = const.tile([128, P], f32)
            hs_sb = const.tile([B0, BPC, D], bf16)

            # sync queue: S-side weights
            nc.sync.dma_start(out=WS_sb[:, 0, :, :], in_=WS_d[0])
            nc.sync.dma_start(out=WS_sb[:, 1:, :, :], in_=WS_d[1:].rearrange("m p k x -> p m k x"))
            nc.sync.dma_start(out=Ww_sb[:], in_=Ww_d)
            nc.sync.dma_start(out=bw_sb[:], in_=bw_d)
            # scalar queue: S-side rhs + V-side rhs
            nc.scalar.dma_start(out=hsT_sb[:], in_=hsT_d)
            nc.scalar.dma_start(out=bSV_sb[:], in_=bSV_d)
            nc.scalar.dma_start(out=hvT_sb[:, 0, :, :], in_=hvT_d[0])
            nc.scalar.dma_start(out=hvT_sb[:, 1:, :, :], in_=hvT_d[1:].rearrange("s p k x -> p s k x"))
            # gpsimd queue: V-side weights + tail tensors
            nc.gpsimd.dma_start(out=WV_sb[:, 0, :, :], in_=WV_d[0])
            nc.gpsimd.dma_start(out=WV_sb[:, 1:, :, :], in_=WV_d[1:].rearrange("m p k x -> p m k x"))
            nc.gpsimd.dma_start(out=mask_sb[:], in_=mask_d)
            nc.gpsimd.dma_start(out=hs_sb[:], in_=hs_d)

            ident = const.tile([128, 128], f32)
            make_identity(nc, ident[:])
            onesP = const.tile([1, P], bf16)
            nc.vector.memset(onesP[:], 1.0)

            VT_sb = const.tile([128, BPC, NC, 128], bf16)
            ST_dup = const.tile([128, NC, P, 2], bf16)

            # ---- helpers -------------------------------------------------
            def proj_S(mc):
                # S chunk: ST'[d, packed(slot, n)] for all slots
                ps_s = psp.tile([128, P], f32, tag="ps")
                for kc in range(NC):
                    nc.tensor.matmul(
                        ps_s[:],
                        WS_sb[:, mc, kc, :],
                        hsT_sb[:, kc, :],
                        start=(kc == 0),
                        stop=False,
                    )
                # + (b_S + b_V) broadcast along packed cols: rank-1 K=1 matmul
                nc.tensor.matmul(
                    ps_s[:],
                    bSV_sb[0:1, mc * 128 : (mc + 1) * 128],
                    onesP[0:1, :],
                    start=False,
                    stop=True,
                )
                # ST_dup[d, mc, p, 2] <- ps_s duplicated over pair axis; for
                # chunk 0 split slot 0 out so the head of the ACT chain isn't
                # gated on the full-P cast
                if mc == 0:
                    nc.vector.tensor_copy(
                        ST_dup[:, mc, 0 : offs[1], :],
                        ps_s[:, 0 : offs[1]]
                        .unsqueeze(2)
                        .broadcast_to([128, bounds[0], 2]),
                    )
                    nc.vector.tensor_copy(
                        ST_dup[:, mc, offs[1] :, :],
                        ps_s[:, offs[1] :]
                        .unsqueeze(2)
                        .broadcast_to([128, P - offs[1], 2]),
                    )
                else:
                    nc.vector.tensor_copy(
                        ST_dup[:, mc, :, :],
                        ps_s[:].unsqueeze(2).broadcast_to([128, P, 2]),
                    )

            def proj_V(mc, k0, k1):
                # V chunk for slots [k0, k1): slots packed in the rhs free
                # dim -> single psum accumulation group
                pv_t = pvp.tile([128, BPC, 128], f32, tag="pv")
                for kc in range(NC):
                    nc.tensor.matmul(
                        pv_t[:, k0:k1, :],
                        WV_sb[:, mc, kc, :],
                        hvT_sb[:, k0:k1, kc, :],
                        start=(kc == 0),
                        stop=(kc == NC - 1),
                    )
                nc.vector.tensor_copy(VT_sb[:, k0:k1, mc, :], pv_t[:, k0:k1, :])

            def ep_add(k, c, ep_slice, n0, n1):
                """e_pre = VT (+bcast over n) + ST' (+bcast over t-pairs)
                for slot k chunk c, rows [n0,n1), into ep_slice."""
                nn = n1 - n0
                nc.vector.tensor_add(
                    ep_slice.rearrange("p n (t two) -> p n t two", two=2),
                    VT_sb[:, k, c, :]
                    .rearrange("p (t two) -> p t two", two=2)
                    .unsqueeze(1)
                    .broadcast_to([128, nn, 64, 2]),
                    ST_dup[:, c, offs[k] + n0 : offs[k] + n1, :]
                    .unsqueeze(2)
                    .broadcast_to([128, nn, 64, 2]),
                )

            def beta_mms(k, c, eb, beta_big, n0, n1):
                for n in range(n0, n1):
                    nc.tensor.matmul(
                        beta_big[:, c, n : n + 1],
                        eb[:, c, n, :],
                        Ww_sb[:, c : c + 1],
                        start=True,
                        stop=True,
                    )

            def add_tanh(k, c, eb):
                """add (DVE 2x) -> tanh (ACT) for one chunk; betas deferred."""
                bk = bounds[k]
                ep = eprep.tile([128, B0, 128], bf16, tag="ep")
                ep_add(k, c, ep[:, 0:bk, :], 0, bk)
                nc.scalar.activation(
                    eb[:, c, :, :], ep[:, 0:bk, :], mybir.ActivationFunctionType.Tanh
                )

            def add_tanh_head(k, c, eb, beta_big):
                """First granule: add/tanh split by n-halves so the ACT chain
                starts on a half-size dependency; betas emitted inline (they
                are the first PE work after projections)."""
                bk = bounds[k]
                h = max(1, bk // 2)
                ep = eprep.tile([128, B0, 128], bf16, tag="ep")
                for n0, n1 in ((0, h), (h, bk)):
                    if n1 <= n0:
                        continue
                    ep_add(k, c, ep[:, n0:n1, :], n0, n1)
                    nc.scalar.activation(
                        eb[:, c, n0:n1, :],
                        ep[:, n0:n1, :],
                        mybir.ActivationFunctionType.Tanh,
                    )

            def granule2(k, c0, eb, beta_big, tail=False):
                """2-chunk granule: two adds, one tanh, betas for both chunks.
                tail=True splits the second chunk's tanh by n-halves so the
                final beta matmuls and softmax start earlier."""
                bk = bounds[k]
                ep = eprep2.tile([128, 2, bounds[1], 128], bf16, tag="ep2")
                ep_add(k, c0, ep[:, 0, 0:bk, :], 0, bk)
                ep_add(k, c0 + 1, ep[:, 1, 0:bk, :], 0, bk)
                if not tail:
                    nc.scalar.activation(
                        eb[:, c0 : c0 + 2, :, :],
                        ep[:, :, 0:bk, :],
                        mybir.ActivationFunctionType.Tanh,
                    )
                    beta_mms(k, c0, eb, beta_big, 0, bk)
                    beta_mms(k, c0 + 1, eb, beta_big, 0, bk)
                else:
                    h = max(1, bk // 2)
                    nc.scalar.activation(
                        eb[:, c0, :, :],
                        ep[:, 0, 0:bk, :],
                        mybir.ActivationFunctionType.Tanh,
                    )
                    beta_mms(k, c0, eb, beta_big, 0, bk)
                    nc.scalar.activation(
                        eb[:, c0 + 1, 0:h, :],
                        ep[:, 1, 0:h, :],
                        mybir.ActivationFunctionType.Tanh,
                    )
                    beta_mms(k, c0 + 1, eb, beta_big, 0, h)
                    nc.scalar.activation(
                        eb[:, c0 + 1, h:bk, :],
                        ep[:, 1, h:bk, :],
                        mybir.ActivationFunctionType.Tanh,
                    )
                    beta_mms(k, c0 + 1, eb, beta_big, h, bk)

            # per-slot softmax state: chunk-pair sums pulled out of PSUM as
            # soon as each half of beta is complete
            s2_tiles = {}

            def beta_pair_copy(k, half, beta_big):
                bk = bounds[k]
                if half == 0:
                    s2_tiles[k] = softp.tile([128, 2, 2, B0], f32, tag="s2c", name=f"s2c{k}")
                nc.vector.tensor_copy(
                    s2_tiles[k][:, half, :, 0:bk],
                    beta_big[:, 2 * half : 2 * half + 2, :],
                )

            def softmax_final(k):
                bk = bounds[k]
                s2c = s2_tiles[k]
                s2 = softp.tile([128, 2, B0], f32, tag="s2")
                nc.vector.tensor_add(
                    s2[:, :, 0:bk], s2c[:, 0, :, 0:bk], s2c[:, 1, :, 0:bk]
                )
                qa = softp.tile([128, B0], f32, tag="qa")
                # qa = (s01 + b_w) + s23
                nc.vector.scalar_tensor_tensor(
                    qa[:, 0:bk],
                    s2[:, 0, 0:bk],
                    bw_sb[:],
                    s2[:, 1, 0:bk],
                    op0=mybir.AluOpType.add,
                    op1=mybir.AluOpType.add,
                )
                # exp directly on qa: pad positions are killed in the
                # numerator by the zeroed h_s rows and in the denominator by
                # the mask mult
                t1 = softp.tile([128, B0], f32, tag="t1")
                nc.scalar.activation(
                    t1[:, 0:bk], qa[:, 0:bk], mybir.ActivationFunctionType.Exp
                )
                q = softp.tile([128, B0], f32, tag="q1")
                nc.vector.tensor_mul(
                    q[:, 0:bk], t1[:, 0:bk], mask_sb[:, offs[k] : offs[k] + bk]
                )
                Qs = softp.tile([128, 1], f32, tag="Z1")
                nc.vector.tensor_reduce(
                    Qs[:], q[:, 0:bk], mybir.AxisListType.X, mybir.AluOpType.add
                )
                recip = softp.tile([128, 1], f32, tag="recip")
                nc.vector.reciprocal(recip[:], Qs[:])
                # ---- out[k] = (t1 @ h_s_masked[k]) * recip ----
                qT_ps = pqtp.tile([B0, 128], f32, tag="qt")
                nc.tensor.transpose(qT_ps[0:bk, :], t1[:, 0:bk], ident[:])
                qT = softp.tile([B0, 128], bf16, tag="qTs")
                nc.vector.tensor_copy(qT[0:bk, :], qT_ps[0:bk, :])
                out_ps = pfinp.tile([128, D], f32, tag="out")
                out_sb = outp.tile([128, D], bf16, tag="osb")
                if k == BPC - 1:
                    # tail: split matmul/scale by D-halves so copy and DMA
                    # overlap the second half's matmul
                    for hf in range(2):
                        cs2 = slice(hf * (D // 2), (hf + 1) * (D // 2))
                        nc.tensor.matmul(
                            out_ps[:, cs2],
                            qT[0:bk, :],
                            hs_sb[0:bk, k, cs2],
                            start=True,
                            stop=True,
                        )
                        nc.vector.tensor_scalar_mul(
                            out_sb[:, cs2], out_ps[:, cs2], recip[:]
                        )
                        nc.sync.dma_start(out=out_d[k][:, cs2], in_=out_sb[:, cs2])
                else:
                    nc.tensor.matmul(
                        out_ps[:], qT[0:bk, :], hs_sb[0:bk, k, :], start=True, stop=True
                    )
                    nc.vector.tensor_scalar_mul(out_sb[:], out_ps[:], recip[:])
                    nc.sync.dma_start(out=out_d[k], in_=out_sb[:])

            # ---- slot 0 interleaved with projections; beta matmuls are
            # emitted one chunk late so the PE queue never blocks a
            # projection behind tanh-gated work ----
            slot_tiles = {}

            def alloc_slot(k):
                slot_tiles[k] = (
                    ebigp.tile(
                        [128, NC, bounds[k], 128], bf16, tag=f"e{k}", name=f"eb{k}"
                    ),
                    pbetap.tile(
                        [128, NC, bounds[k]], f32, tag="beta", name=f"bb{k}"
                    ),
                )

            for mc in range(NC):
                proj_S(mc)
                if mc == 0:
                    proj_V(0, 0, 1)
                    alloc_slot(0)
                    add_tanh_head(0, 0, *slot_tiles[0])
                else:
                    proj_V(mc, 0, BPC)
                    beta_mms(0, mc - 1, *slot_tiles[0], 0, bounds[0])
                    if mc == 2:
                        beta_pair_copy(0, 0, slot_tiles[0][1])
                    add_tanh(0, mc, slot_tiles[0][0])
            proj_V(0, 1, BPC)
            beta_mms(0, NC - 1, *slot_tiles[0], 0, bounds[0])
            beta_pair_copy(0, 1, slot_tiles[0][1])

            # ---- remaining slots, softmax/final pipelined one slot late ----
            for k in range(1, BPC):
                alloc_slot(k)
                eb, beta_big = slot_tiles[k]
                last = k == BPC - 1
                if not last:
                    granule2(k, 0, eb, beta_big)
                    beta_pair_copy(k, 0, beta_big)
                    softmax_final(k - 1)
                    granule2(k, 2, eb, beta_big)
                    beta_pair_copy(k, 1, beta_big)
                else:
                    # last slot: softmax first so its DVE ops don't sit
                    # between this slot's adds (the end is the critical tail)
                    softmax_final(k - 1)
                    granule2(k, 0, eb, beta_big)
                    beta_pair_copy(k, 0, beta_big)
                    granule2(k, 2, eb, beta_big, tail=True)
                    beta_pair_copy(k, 1, beta_big)
            softmax_final(BPC - 1)

    nc.compile()
    return nc


def _get_nc(bounds):
    key = tuple(bounds)
    if key not in _CACHE:
        _CACHE[key] = _build(list(bounds))
    return _CACHE[key]


def _plan(lengths):
    """Sort batches by length desc; slot k on core c <- sorted rank 8k+c.
    Returns (order, bounds)."""
    lengths = np.asarray(lengths).reshape(-1)
    order = np.argsort(-lengths, kind="stable")
    bounds = [int(lengths[order[NCORES * k]]) for k in range(BPC)]
    return order, bounds


def _make_in_maps(order, bounds, h_s, h_v, lengths, W_S, b_S, W_V, b_V, W_w, b_w):
    f32 = np.float32
    h_s = np.asarray(h_s, dtype=f32)
    h_v = np.asarray(h_v, dtype=f32)
    lengths = np.asarray(lengths).reshape(-1)
    offs = np.concatenate([[0], np.cumsum(bounds)]).astype(int)
    P = int(offs[-1])
    B0 = bounds[0]

    # weights, chunked + cast once (shared across cores); mc outermost
    WS = np.ascontiguousarray(
        np.asarray(W_S, f32).reshape(NC, 128, NC, 128).transpose(2, 1, 0, 3)
    )  # [mc, p, kc, 128]
    WV = np.ascontiguousarray(
        np.asarray(W_V, f32).reshape(NC, 128, NC, 128).transpose(2, 1, 0, 3)
    )
    Ww = np.ascontiguousarray(np.asarray(W_w, f32).reshape(NC, 128).T)
    bSV = (np.asarray(b_S, f32) + np.asarray(b_V, f32)).reshape(1, D)
    bw_rep = np.full((128, 1), f32(np.asarray(b_w).reshape(-1)[0]), dtype=f32)

    try:
        import ml_dtypes

        bf16 = ml_dtypes.bfloat16
    except ImportError:
        import jax.numpy as jnp

        bf16 = jnp.bfloat16

    def to_bf16(x):
        return np.asarray(x, dtype=bf16)

    WS_b = to_bf16(WS)
    WV_b = to_bf16(WV)
    Ww_b = to_bf16(Ww)
    bSV_b = to_bf16(bSV)

    in_maps = []
    for core in range(NCORES):
        batches = [int(order[NCORES * k + core]) for k in range(BPC)]
        hv_c = h_v[batches]  # (BPC, T, D)
        hvT = np.ascontiguousarray(
            hv_c.reshape(BPC, T, NC, 128).transpose(0, 3, 2, 1)
        )  # (slot, 128p, kc, t)
        hsT = np.zeros((128, NC, P), dtype=f32)
        hs_r = np.zeros((B0, BPC, D), dtype=f32)  # (n, slot, D), masked rows 0
        mask_bc = np.zeros((128, P), dtype=f32)
        for k, b in enumerate(batches):
            L = int(lengths[b])
            bk = bounds[k]
            Lk = min(L, bk)
            hk = h_s[b, :Lk]  # (Lk, D)
            hsT[:, :, offs[k] : offs[k] + Lk] = hk.reshape(Lk, NC, 128).transpose(
                2, 1, 0
            )
            hs_r[:Lk, k, :] = hk
            mask_bc[:, offs[k] : offs[k] + Lk] = 1.0
        in_maps.append(
            {
                "hvT": to_bf16(hvT),
                "hsT": to_bf16(hsT),
                "hs": to_bf16(hs_r),
                "WS": WS_b,
                "WV": WV_b,
                "Ww": Ww_b,
                "bSV": bSV_b,
                "bw": bw_rep,
                "mask": mask_bc,
            }
        )
    return in_maps


def run(inputs: dict, trace: bool = False):
    """Run on 8 NeuronCores; returns (output, BassKernelResults)."""
    from concourse import bass_utils

    order, bounds = _plan(inputs["lengths"])
    nc = _get_nc(bounds)
    in_maps = _make_in_maps(order, bounds, **inputs)
    res = bass_utils.run_bass_kernel_spmd(
        nc, in_maps, core_ids=list(range(NCORES)), trace=trace
    )
    full = np.zeros((B, T, D), dtype=np.float32)
    for core in range(NCORES):
        o = np.asarray(res.results[core]["out"], dtype=np.float32)
        for k in range(BPC):
            full[int(order[NCORES * k + core])] = o[k]
    return full, res


def kernel(**inputs) -> np.ndarray:
    out, _ = run(inputs, trace=False)
    return out


# revision 7
# speedup vs baseline: 1.3950x; 1.0116x over previous
"""Trainium2 Bass kernel for InteractorwoLSTM additive attention.

out[b,t,:] = alpha[b,t,:] @ h_s[b]  with
  beta[b,t,n] = W_w . tanh(h_s[b,n]@W_S + b_S + h_v[b,t]@W_V + b_V) + b_w
  alpha = masked-softmax(beta) per reference semantics.

v4 design: length-truncated slots + JIT DMA + tight ramp/tail.

Positions n >= lengths[b] never influence the output (the reference's
masked-softmax renormalization cancels them), so the tanh/add/beta work
for those positions is skipped. lengths are known on the host before
compile, so the program is built for the actual length profile:
batches sorted by length desc; slot k on core c holds sorted rank
8k+c; slot k's n-bound = L(rank 8k) (SPMD shares one program). For the
seed-0 data: bounds [30,17,12,8] -> 67 packed n-columns vs 120
(~44% less work on ACT -- the bottleneck engine at 1 elem/cycle/lane).

v4 over v3 (v3 = 59.8us, ACT busy 34.5us, ramp-to-first-tanh 16us,
tail 9us):
  - weight/input DRAM layouts are chunk-outermost so each DMA piece is
    a contiguous 1KB-per-partition run; pieces are ordered/queued so
    the first granule's deps (WS0/hsT/WV0/hvT0) land first.
  - slot-0 beta matmuls are emitted one chunk late so the PE queue
    never blocks projections behind tanh-gated work.
  - softmax chunk-sums read beta PSUM in c01/c23 halves as soon as
    each half is done -> the tail only carries the c23 copy.
  - last granule and final matmul/copy split for a shorter tail.
  - dummy tanh at the top forces the ACT table load (~2.7us) under the
    initial DMAs.
"""

import numpy as np

B, T, N = 32, 128, 30
D = 512
NCORES = 8
BPC = B // NCORES  # batch slots per core
NC = D // 128  # 4 chunks of 128 along D

_CACHE = {}


def _build(bounds):
    import concourse.bacc as bacc
    import concourse.tile as tile
    from concourse import mybir
    from concourse.masks import make_identity

    f32 = mybir.dt.float32
    bf16 = mybir.dt.bfloat16

    offs = [0]
    for b in bounds:
        offs.append(offs[-1] + b)
    P = offs[-1]
    B0 = bounds[0]

    nc = bacc.Bacc(
        "TRN2",
        target_bir_lowering=False,
        debug=False,
        enable_asserts=True,
        num_devices=NCORES,
    )

    # ---- DRAM I/O (host-prepped layouts, chunk-outermost for DMA) ----
    hvT_d = nc.dram_tensor("hvT", [BPC, 128, NC, 128], bf16, kind="ExternalInput").ap()
    hsT_d = nc.dram_tensor("hsT", [128, NC, P], bf16, kind="ExternalInput").ap()
    hs_d = nc.dram_tensor("hs", [B0, BPC, D], bf16, kind="ExternalInput").ap()
    WS_d = nc.dram_tensor("WS", [NC, 128, NC, 128], bf16, kind="ExternalInput").ap()
    WV_d = nc.dram_tensor("WV", [NC, 128, NC, 128], bf16, kind="ExternalInput").ap()
    Ww_d = nc.dram_tensor("Ww", [128, NC], bf16, kind="ExternalInput").ap()
    bSV_d = nc.dram_tensor("bSV", [1, D], bf16, kind="ExternalInput").ap()
    bw_d = nc.dram_tensor("bw", [128, 1], f32, kind="ExternalInput").ap()
    mask_d = nc.dram_tensor("mask", [128, P], f32, kind="ExternalInput").ap()
    out_d = nc.dram_tensor("out", [BPC, T, D], bf16, kind="ExternalOutput").ap()

    with tile.TileContext(nc) as tc:
        with (
            tc.tile_pool(name="const", bufs=1) as const,
            tc.tile_pool(name="epre", bufs=3) as eprep,
            tc.tile_pool(name="epre2", bufs=3) as eprep2,
            tc.tile_pool(name="ebig", bufs=1) as ebigp,
            tc.tile_pool(name="soft", bufs=3) as softp,
            tc.tile_pool(name="outp", bufs=3) as outp,
            tc.tile_pool(name="pv", bufs=2, space="PSUM") as pvp,
            tc.tile_pool(name="ps", bufs=2, space="PSUM") as psp,
            tc.tile_pool(name="pbeta", bufs=2, space="PSUM") as pbetap,
            tc.tile_pool(name="pqt", bufs=1, space="PSUM") as pqtp,
            tc.tile_pool(name="pfin", bufs=1, space="PSUM") as pfinp,
        ):
            # ---- force the exp/tanh ACT table load to start at t=0 so it
            # hides under the weight DMAs (the set covers tanh+exp+copy)
            warm = const.tile([1, 2], f32)
            nc.vector.memset(warm[:], 0.0)
            warm2 = const.tile([1, 2], f32)
            nc.scalar.activation(warm2[:], warm[:], mybir.ActivationFunctionType.Tanh)

            # ---- inputs: each queue's pieces ordered by first use ----
            WS_sb = const.tile([128, NC, NC, 128], bf16)  # [p, mc, kc, 128]
            WV_sb = const.tile([128, NC, NC, 128], bf16)  # [p, mc, kc, 128]
            hvT_sb = const.tile([128, BPC, NC, 128], bf16)  # [p, slot, kc, t]
            hsT_sb = const.tile([128, NC, P], bf16)
            bSV_sb = const.tile([1, D], bf16)
            Ww_sb = const.tile([128, NC], bf16)
            bw_sb = const.tile([128, 1], f32)
            mask_sb = const.tile([128, P], f32)
            hs_sb = const.tile([B0, BPC, D], bf16)

            # each queue's first pieces are exactly the first-granule deps;
            # later pieces arrive progressively per-chunk (per-mc/slot)
            # sync queue: WS0 + hsT gate proj_S(0)
            nc.sync.dma_start(out=WS_sb[:, 0, :, :], in_=WS_d[0])
            nc.sync.dma_start(out=hsT_sb[:], in_=hsT_d)
            for mc in range(1, NC):
                nc.sync.dma_start(out=WS_sb[:, mc, :, :], in_=WS_d[mc])
            nc.sync.dma_start(out=Ww_sb[:], in_=Ww_d)
            nc.sync.dma_start(out=bw_sb[:], in_=bw_d)
            # scalar queue: hvT0 gates proj_V(0, slot0)
            nc.scalar.dma_start(out=hvT_sb[:, 0, :, :], in_=hvT_d[0])
            nc.scalar.dma_start(out=bSV_sb[:], in_=bSV_d)
            for k in range(1, BPC):
                nc.scalar.dma_start(out=hvT_sb[:, k, :, :], in_=hvT_d[k])
            # gpsimd queue: WV0 gates proj_V(0, slot0); tail tensors last
            for mc in range(NC):
                nc.gpsimd.dma_start(out=WV_sb[:, mc, :, :], in_=WV_d[mc])
            nc.gpsimd.dma_start(out=mask_sb[:], in_=mask_d)
            nc.gpsimd.dma_start(out=hs_sb[:], in_=hs_d)

            ident = const.tile([128, 128], f32)
            make_identity(nc, ident[:])
            onesP = const.tile([1, P], bf16)
            nc.vector.memset(onesP[:], 1.0)

            VT_sb = const.tile([128, BPC, NC, 128], bf16)
            ST_dup = const.tile([128, NC, P, 2], bf16)

            # ---- helpers -------------------------------------------------
            def proj_S(mc):
                # S chunk: ST'[d, packed(slot, n)] for all slots
                ps_s = psp.tile([128, P], f32, tag="ps")
                for kc in range(NC):
                    nc.tensor.matmul(
                        ps_s[:],
                        WS_sb[:, mc, kc, :],
                        hsT_sb[:, kc, :],
                        start=(kc == 0),
                        stop=False,
                    )
                # + (b_S + b_V) broadcast along packed cols: rank-1 K=1 matmul
                nc.tensor.matmul(
                    ps_s[:],
                    bSV_sb[0:1, mc * 128 : (mc + 1) * 128],
                    onesP[0:1, :],
                    start=False,
                    stop=True,
                )
                # ST_dup[d, mc, p, 2] <- ps_s duplicated over pair axis; for
                # chunk 0 split slot 0 out so the head of the ACT chain isn't
                # gated on the full-P cast
                if mc == 0:
                    nc.vector.tensor_copy(
                        ST_dup[:, mc, 0 : offs[1], :],
                        ps_s[:, 0 : offs[1]]
                        .unsqueeze(2)
                        .broadcast_to([128, bounds[0], 2]),
                    )
                    nc.vector.tensor_copy(
                        ST_dup[:, mc, offs[1] :, :],
                        ps_s[:, offs[1] :]
                        .unsqueeze(2)
                        .broadcast_to([128, P - offs[1], 2]),
                    )
                else:
                    nc.vector.tensor_copy(
                        ST_dup[:, mc, :, :],
                        ps_s[:].unsqueeze(2).broadcast_to([128, P, 2]),
                    )

            def proj_V(mc, k0, k1):
                # V chunk for slots [k0, k1): slots packed in the rhs free
                # dim -> single psum accumulation group
                pv_t = pvp.tile([128, BPC, 128], f32, tag="pv")
                for kc in range(NC):
                    nc.tensor.matmul(
                        pv_t[:, k0:k1, :],
                        WV_sb[:, mc, kc, :],
                        hvT_sb[:, k0:k1, kc, :],
                        start=(kc == 0),
                        stop=(kc == NC - 1),
                    )
                nc.vector.tensor_copy(VT_sb[:, k0:k1, mc, :], pv_t[:, k0:k1, :])

            def ep_add(k, c, ep_slice, n0, n1):
                """e_pre = VT (+bcast over n) + ST' (+bcast over t-pairs)
                for slot k chunk c, rows [n0,n1), into ep_slice."""
                nn = n1 - n0
                nc.vector.tensor_add(
                    ep_slice.rearrange("p n (t two) -> p n t two", two=2),
                    VT_sb[:, k, c, :]
                    .rearrange("p (t two) -> p t two", two=2)
                    .unsqueeze(1)
                    .broadcast_to([128, nn, 64, 2]),
                    ST_dup[:, c, offs[k] + n0 : offs[k] + n1, :]
                    .unsqueeze(2)
                    .broadcast_to([128, nn, 64, 2]),
                )

            def beta_mms(k, c, eb, beta_big, n0, n1):
                for n in range(n0, n1):
                    nc.tensor.matmul(
                        beta_big[:, c, n : n + 1],
                        eb[:, c, n, :],
                        Ww_sb[:, c : c + 1],
                        start=True,
                        stop=True,
                    )

            def add_tanh(k, c, eb):
                """add (DVE 2x) -> tanh (ACT) for one chunk; betas deferred."""
                bk = bounds[k]
                ep = eprep.tile([128, B0, 128], bf16, tag="ep")
                ep_add(k, c, ep[:, 0:bk, :], 0, bk)
                nc.scalar.activation(
                    eb[:, c, :, :], ep[:, 0:bk, :], mybir.ActivationFunctionType.Tanh
                )

            def add_tanh_head(k, c, eb, beta_big):
                """First granule: add/tanh split by n-halves so the ACT chain
                starts on a half-size dependency; betas emitted inline (they
                are the first PE work after projections)."""
                bk = bounds[k]
                h = max(1, bk // 2)
                ep = eprep.tile([128, B0, 128], bf16, tag="ep")
                for n0, n1 in ((0, h), (h, bk)):
                    if n1 <= n0:
                        continue
                    ep_add(k, c, ep[:, n0:n1, :], n0, n1)
                    nc.scalar.activation(
                        eb[:, c, n0:n1, :],
                        ep[:, n0:n1, :],
                        mybir.ActivationFunctionType.Tanh,
                    )

            def granule2(k, c0, eb, beta_big, tail=False):
                """2-chunk granule: two adds, one tanh, betas for both chunks.
                tail=True splits the second chunk's tanh by n-halves so the
                final beta matmuls and softmax start earlier."""
                bk = bounds[k]
                ep = eprep2.tile([128, 2, bounds[1], 128], bf16, tag="ep2")
                ep_add(k, c0, ep[:, 0, 0:bk, :], 0, bk)
                ep_add(k, c0 + 1, ep[:, 1, 0:bk, :], 0, bk)
                if not tail:
                    nc.scalar.activation(
                        eb[:, c0 : c0 + 2, :, :],
                        ep[:, :, 0:bk, :],
                        mybir.ActivationFunctionType.Tanh,
                    )
                    beta_mms(k, c0, eb, beta_big, 0, bk)
                    beta_mms(k, c0 + 1, eb, beta_big, 0, bk)
                else:
                    h = max(1, bk // 2)
                    nc.scalar.activation(
                        eb[:, c0, :, :],
                        ep[:, 0, 0:bk, :],
                        mybir.ActivationFunctionType.Tanh,
                    )
                    beta_mms(k, c0, eb, beta_big, 0, bk)
                    nc.scalar.activation(
                        eb[:, c0 + 1, 0:h, :],
                        ep[:, 1, 0:h, :],
                        mybir.ActivationFunctionType.Tanh,
                    )
                    beta_mms(k, c0 + 1, eb, beta_big, 0, h)
                    nc.scalar.activation(
                        eb[:, c0 + 1, h:bk, :],
                        ep[:, 1, h:bk, :],
                        mybir.ActivationFunctionType.Tanh,
                    )
                    beta_mms(k, c0 + 1, eb, beta_big, h, bk)

            # per-slot softmax state: chunk-pair sums pulled out of PSUM as
            # soon as each half of beta is complete
            s2_tiles = {}

            def beta_pair_copy(k, half, beta_big):
                bk = bounds[k]
                if half == 0:
                    s2_tiles[k] = softp.tile([128, 2, 2, B0], f32, tag="s2c", name=f"s2c{k}")
                nc.vector.tensor_copy(
                    s2_tiles[k][:, half, :, 0:bk],
                    beta_big[:, 2 * half : 2 * half + 2, :],
                )

            def softmax_final(k):
                bk = bounds[k]
                s2c = s2_tiles[k]
                s2 = softp.tile([128, 2, B0], f32, tag="s2")
                nc.vector.tensor_add(
                    s2[:, :, 0:bk], s2c[:, 0, :, 0:bk], s2c[:, 1, :, 0:bk]
                )
                qa = softp.tile([128, B0], f32, tag="qa")
                # qa = (s01 + b_w) + s23
                nc.vector.scalar_tensor_tensor(
                    qa[:, 0:bk],
                    s2[:, 0, 0:bk],
                    bw_sb[:],
                    s2[:, 1, 0:bk],
                    op0=mybir.AluOpType.add,
                    op1=mybir.AluOpType.add,
                )
                # exp directly on qa: pad positions are killed in the
                # numerator by the zeroed h_s rows and in the denominator by
                # the mask mult
                t1 = softp.tile([128, B0], f32, tag="t1")
                nc.scalar.activation(
                    t1[:, 0:bk], qa[:, 0:bk], mybir.ActivationFunctionType.Exp
                )
                q = softp.tile([128, B0], f32, tag="q1")
                nc.vector.tensor_mul(
                    q[:, 0:bk], t1[:, 0:bk], mask_sb[:, offs[k] : offs[k] + bk]
                )
                Qs = softp.tile([128, 1], f32, tag="Z1")
                nc.vector.tensor_reduce(
                    Qs[:], q[:, 0:bk], mybir.AxisListType.X, mybir.AluOpType.add
                )
                recip = softp.tile([128, 1], f32, tag="recip")
                nc.vector.reciprocal(recip[:], Qs[:])
                # ---- out[k] = (t1 @ h_s_masked[k]) * recip ----
                qT_ps = pqtp.tile([B0, 128], f32, tag="qt")
                nc.tensor.transpose(qT_ps[0:bk, :], t1[:, 0:bk], ident[:])
                qT = softp.tile([B0, 128], bf16, tag="qTs")
                nc.vector.tensor_copy(qT[0:bk, :], qT_ps[0:bk, :])
                out_ps = pfinp.tile([128, D], f32, tag="out")
                out_sb = outp.tile([128, D], bf16, tag="osb")
                if k == BPC - 1:
                    # tail: split matmul/scale by D-halves so copy and DMA
                    # overlap the second half's matmul
                    for hf in range(2):
                        cs2 = slice(hf * (D // 2), (hf + 1) * (D // 2))
                        nc.tensor.matmul(
                            out_ps[:, cs2],
                            qT[0:bk, :],
                            hs_sb[0:bk, k, cs2],
                            start=True,
                            stop=True,
                        )
                        nc.vector.tensor_scalar_mul(
                            out_sb[:, cs2], out_ps[:, cs2], recip[:]
                        )
                        nc.sync.dma_start(out=out_d[k][:, cs2], in_=out_sb[:, cs2])
                else:
                    nc.tensor.matmul(
                        out_ps[:], qT[0:bk, :], hs_sb[0:bk, k, :], start=True, stop=True
                    )
                    nc.vector.tensor_scalar_mul(out_sb[:], out_ps[:], recip[:])
                    nc.sync.dma_start(out=out_d[k], in_=out_sb[:])

            # ---- slot 0 interleaved with projections; beta matmuls are
            # emitted one chunk late so the PE queue never blocks a
            # projection behind tanh-gated work ----
            slot_tiles = {}

            def alloc_slot(k):
                slot_tiles[k] = (
                    ebigp.tile(
                        [128, NC, bounds[k], 128], bf16, tag=f"e{k}", name=f"eb{k}"
                    ),
                    pbetap.tile(
                        [128, NC, bounds[k]], f32, tag="beta", name=f"bb{k}"
                    ),
                )

            for mc in range(NC):
                proj_S(mc)
                if mc == 0:
                    proj_V(0, 0, 1)
                    alloc_slot(0)
                    add_tanh_head(0, 0, *slot_tiles[0])
                else:
                    proj_V(mc, 0, BPC)
                    beta_mms(0, mc - 1, *slot_tiles[0], 0, bounds[0])
                    if mc == 2:
                        beta_pair_copy(0, 0, slot_tiles[0][1])
                    add_tanh(0, mc, slot_tiles[0][0])
            proj_V(0, 1, BPC)
            beta_mms(0, NC - 1, *slot_tiles[0], 0, bounds[0])
            beta_pair_copy(0, 1, slot_tiles[0][1])

            # ---- remaining slots, softmax/final pipelined one slot late ----
            for k in range(1, BPC):
                alloc_slot(k)
                eb, beta_big = slot_tiles[k]
                last = k == BPC - 1
                if not last:
                    granule2(k, 0, eb, beta_big)
                    beta_pair_copy(k, 0, beta_big)
                    softmax_final(k - 1)
                    granule2(k, 2, eb, beta_big)
                    beta_pair_copy(k, 1, beta_big)
                else:
                    # last slot: softmax first so its DVE ops don't sit
                    # between this slot's adds (the end is the critical tail)
                    softmax_final(k - 1)
                    granule2(k, 0, eb, beta_big)
                    beta_pair_copy(k, 0, beta_big)
                    granule2(k, 2, eb, beta_big, tail=True)
                    beta_pair_copy(k, 1, beta_big)
            softmax_final(BPC - 1)

    nc.compile()
    return nc


def _get_nc(bounds):
    key = tuple(bounds)
    if key not in _CACHE:
        _CACHE[key] = _build(list(bounds))
    return _CACHE[key]


def _plan(lengths):
    """Sort batches by length desc; slot k on core c <- sorted rank 8k+c.
    Returns (order, bounds)."""
    lengths = np.asarray(lengths).reshape(-1)
    order = np.argsort(-lengths, kind="stable")
    bounds = [int(lengths[order[NCORES * k]]) for k in range(BPC)]
    return order, bounds


def _make_in_maps(order, bounds, h_s, h_v, lengths, W_S, b_S, W_V, b_V, W_w, b_w):
    f32 = np.float32
    h_s = np.asarray(h_s, dtype=f32)
    h_v = np.asarray(h_v, dtype=f32)
    lengths = np.asarray(lengths).reshape(-1)
    offs = np.concatenate([[0], np.cumsum(bounds)]).astype(int)
    P = int(offs[-1])
    B0 = bounds[0]

    # weights, chunked + cast once (shared across cores); mc outermost
    WS = np.ascontiguousarray(
        np.asarray(W_S, f32).reshape(NC, 128, NC, 128).transpose(2, 1, 0, 3)
    )  # [mc, p, kc, 128]
    WV = np.ascontiguousarray(
        np.asarray(W_V, f32).reshape(NC, 128, NC, 128).transpose(2, 1, 0, 3)
    )
    Ww = np.ascontiguousarray(np.asarray(W_w, f32).reshape(NC, 128).T)
    bSV = (np.asarray(b_S, f32) + np.asarray(b_V, f32)).reshape(1, D)
    bw_rep = np.full((128, 1), f32(np.asarray(b_w).reshape(-1)[0]), dtype=f32)

    try:
        import ml_dtypes

        bf16 = ml_dtypes.bfloat16
    except ImportError:
        import jax.numpy as jnp

        bf16 = jnp.bfloat16

    def to_bf16(x):
        return np.asarray(x, dtype=bf16)

    WS_b = to_bf16(WS)
    WV_b = to_bf16(WV)
    Ww_b = to_bf16(Ww)
    bSV_b = to_bf16(bSV)

    in_maps = []
    for core in range(NCORES):
        batches = [int(order[NCORES * k + core]) for k in range(BPC)]
        hv_c = h_v[batches]  # (BPC, T, D)
        hvT = np.ascontiguousarray(
            hv_c.reshape(BPC, T, NC, 128).transpose(0, 3, 2, 1)
        )  # (slot, 128p, kc, t)
        hsT = np.zeros((128, NC, P), dtype=f32)
        hs_r = np.zeros((B0, BPC, D), dtype=f32)  # (n, slot, D), masked rows 0
        mask_bc = np.zeros((128, P), dtype=f32)
        for k, b in enumerate(batches):
            L = int(lengths[b])
            bk = bounds[k]
            Lk = min(L, bk)
            hk = h_s[b, :Lk]  # (Lk, D)
            hsT[:, :, offs[k] : offs[k] + Lk] = hk.reshape(Lk, NC, 128).transpose(
                2, 1, 0
            )
            hs_r[:Lk, k, :] = hk
            mask_bc[:, offs[k] : offs[k] + Lk] = 1.0
        in_maps.append(
            {
                "hvT": to_bf16(hvT),
                "hsT": to_bf16(hsT),
                "hs": to_bf16(hs_r),
                "WS": WS_b,
                "WV": WV_b,
                "Ww": Ww_b,
                "bSV": bSV_b,
                "bw": bw_rep,
                "mask": mask_bc,
            }
        )
    return in_maps


def run(inputs: dict, trace: bool = False):
    """Run on 8 NeuronCores; returns (output, BassKernelResults)."""
    from concourse import bass_utils

    order, bounds = _plan(inputs["lengths"])
    nc = _get_nc(bounds)
    in_maps = _make_in_maps(order, bounds, **inputs)
    res = bass_utils.run_bass_kernel_spmd(
        nc, in_maps, core_ids=list(range(NCORES)), trace=trace
    )
    full = np.zeros((B, T, D), dtype=np.float32)
    for core in range(NCORES):
        o = np.asarray(res.results[core]["out"], dtype=np.float32)
        for k in range(BPC):
            full[int(order[NCORES * k + core])] = o[k]
    return full, res


def kernel(**inputs) -> np.ndarray:
    out, _ = run(inputs, trace=False)
    return out
